# revision 1
# baseline (speedup 1.0000x reference)
"""Trainium2 Bass kernel for the LSTM caption decoder (nn_Decoder_62483184222858).

Math (per reference):
    emb = embed_W[captions]                      # [B, T, E]
    h0 = feature, c0 = 0
    for t in 0..T-2:
        gates = x_t @ W_ih.T + h @ W_hh.T + (b_ih + b_hh)   # [B, 4H] order i,f,g,o
        i, f, o = sigmoid(...); g = tanh(g)
        c = f*c + i*g
        h = o*tanh(c) + feature                   # emitted output AND carried state
    logits = outs @ lin_W.T + lin_b               # [B, T-1, V]

Strategy: data-parallel over 8 NeuronCores (64 batch rows each).
Device-side:
  phase A: token_proj[v] = embed_W[v] @ W_ih.T + (b_ih+b_hh)  -> DRAM [V, 4H]
  loop:    x_proj gathered by token id (indirect DMA);
           PE streams h @ W_hh.T into PSUM per gate; DVE adds x_proj;
           ACT sigmoid/tanh; DVE c/h updates; PE transposes h -> hT
           (stationary operand of next step); hT stashed to DRAM (bf16).
           The o-gate + c/h tail is processed in two hidden-halves so the
           first hT chunks are ready ~1.5us after the last gate matmul.
  phase C: logits = outsT.T @ lin_W.T + lin_b at M=128, written per 2 steps.

NOTE: TRN2 engine instructions support ONE semaphore wait each; the Bacc
layer (nc.compile()) legalizes multi-wait joins via InstEventSemaphore.
Build with bacc.Bacc, not raw bass.Bass, or walrus rejects the program
("Too many sync wait commands").
"""

import sys

if "/opt/trn_rl_repo" not in sys.path:
    sys.path.insert(0, "/opt/trn_rl_repo")

import numpy as np
import ml_dtypes

import concourse.bass as bass
import concourse.mybir as mybir
import concourse.tile as tile
from concourse import bacc
from concourse.bass_utils import run_bass_kernel_spmd
from concourse.masks import make_identity

F32 = mybir.dt.float32
BF16 = mybir.dt.bfloat16
I32 = mybir.dt.int32
AF = mybir.ActivationFunctionType

EMBED, HIDDEN, VOCAB = 512, 1024, 1004
B, T = 512, 65
NCORES = 8
BL = B // NCORES          # 64 batch rows per core
TS = T - 1                # 64 time steps
G4 = 4 * HIDDEN           # 4096 gate width
KK_H = HIDDEN // 128      # 8 contraction chunks over hidden
KK_E = EMBED // 128       # 4 contraction chunks over embed
NVT = (VOCAB + 127) // 128  # 8 vocab tiles (last is 108 rows)
HH = HIDDEN // 2          # 512: half-hidden tail granularity

# blob_a (bf16) layout: embWT | WihT
A_EMB = 0
A_WIH = A_EMB + KK_E * VOCAB            # 4016
A_END = A_WIH + KK_E * G4               # 20400
# blob_b (bf16) layout: featT | W_hh gate-major blocks
# W block bi = (gate_stream_idx*2 + half)*KK_H + k, 512 cols each,
# gate stream order [g, i, f, o]
B_FT = 0
B_WHH = KK_H * BL                       # 512
B_END = B_WHH + KK_H * G4               # 33280


def build_program(steps=TS):
    nc = bacc.Bacc("TRN2", target_bir_lowering=False, debug=False)

    blob_a = nc.dram_tensor("blob_a", [128, A_END], BF16, kind="ExternalInput")
    blob_b = nc.dram_tensor("blob_b", [128, B_END], BF16, kind="ExternalInput")
    biasg = nc.dram_tensor("biasg", [1, G4], F32, kind="ExternalInput")
    feat = nc.dram_tensor("feat", [BL, HIDDEN], BF16, kind="ExternalInput")
    caps = nc.dram_tensor("caps", [BL, TS], I32, kind="ExternalInput")
    linWT = nc.dram_tensor("linWT", [128, KK_H * VOCAB], BF16, kind="ExternalInput")
    linb = nc.dram_tensor("linb", [1, VOCAB], F32, kind="ExternalInput")
    out = nc.dram_tensor("out", [BL, TS, VOCAB], F32, kind="ExternalOutput")

    tokp = nc.dram_tensor("tokp", [VOCAB, G4], F32, kind="Internal")
    outsT = nc.dram_tensor("outsT", [KK_H, 128, TS * BL], BF16, kind="Internal")

    with tile.TileContext(nc) as tc:
        _body(nc, tc, steps,
              blob_a.ap(), blob_b.ap(), biasg.ap(), feat.ap(), caps.ap(),
              linWT.ap(), linb.ap(), out.ap(), tokp.ap(), outsT.ap())
    nc.compile()
    return nc


def _body(nc, tc, steps, blob_a, blob_b, biasg, feat, caps, linWT, linb, out,
          tokp, outsT):
    with (
        tc.tile_pool(name="pg", bufs=1) as pg,
        tc.tile_pool(name="pgp", bufs=1, space="PSUM") as pgp,
    ):
        ident = pg.tile([BL, BL], BF16, tag="ident")
        make_identity(nc, ident[:])

        # ================= phase A: token_proj ============================
        with (
            tc.tile_pool(name="pa", bufs=1) as pa,
            tc.tile_pool(name="pap", bufs=1, space="PSUM") as pap,
        ):
            ba = pa.tile([128, A_END], BF16, tag="blob_a")
            nc.sync.dma_start(ba[:], blob_a)
            embWT_sb = ba[:, A_EMB:A_EMB + KK_E * VOCAB]
            WihT_sb = ba[:, A_WIH:A_WIH + KK_E * G4]
            bias_sb = pa.tile([128, G4], F32, tag="bias")
            nc.sync.dma_start(bias_sb[:], biasg.to_broadcast((128, G4)))

            # prefetch later-phase constants (behind phase A's own loads on
            # the same HWDGE ring, so phase A starts ASAP)
            bb = pg.tile([128, B_END], BF16, tag="blob_b")
            nc.sync.dma_start(bb[:, B_FT:B_FT + KK_H * BL],
                              blob_b[:, B_FT:B_FT + KK_H * BL])
            GW = 2 * KK_H * 512  # cols per gate (2 halves x 8 k-chunks)
            for gi in range(4):
                nc.sync.dma_start(
                    bb[:, B_WHH + gi * GW:B_WHH + (gi + 1) * GW],
                    blob_b[:, B_WHH + gi * GW:B_WHH + (gi + 1) * GW])
            W_sb = bb[:, B_WHH:B_WHH + KK_H * G4]
            hT_init = bb[:, B_FT:B_FT + KK_H * BL]
            feat_sb = pg.tile([BL, HIDDEN], BF16, tag="feat")
            nc.sync.dma_start(feat_sb[:], feat)
            caps_sb = pg.tile([BL, TS], I32, tag="caps")
            nc.sync.dma_start(caps_sb[:], caps)
            linWT_sb = pg.tile([128, KK_H * VOCAB], BF16, tag="linWT")
            nc.sync.dma_start(linWT_sb[:], linWT)
            lb_sb = pg.tile([128, VOCAB], F32, tag="lb")
            nc.sync.dma_start(lb_sb[:], linb.to_broadcast((128, VOCAB)))

            for m in range(NVT):
                mrows = min(128, VOCAB - 128 * m)
                for nb in range(G4 // 512):
                    ps = pap.tile([128, 512], F32, tag="tp_ps", bufs=8)
                    for k in range(KK_E):
                        nc.tensor.matmul(
                            ps[:mrows],
                            lhsT=embWT_sb[:, k * VOCAB + 128 * m:
                                          k * VOCAB + 128 * m + mrows],
                            rhs=WihT_sb[:, k * G4 + 512 * nb:
                                        k * G4 + 512 * (nb + 1)],
                            start=(k == 0), stop=(k == KK_E - 1),
                        )
                    sb = pa.tile([128, 512], F32, tag="tp_sb", bufs=8)
                    nc.vector.tensor_add(
                        sb[:mrows], ps[:mrows],
                        bias_sb[:mrows, 512 * nb:512 * (nb + 1)])
                    nc.sync.dma_start(
                        tokp[128 * m:128 * m + mrows, 512 * nb:512 * (nb + 1)],
                        sb[:mrows])

        # ================= phase B: recurrence + in-loop logits ===========
        with (
            tc.tile_pool(name="pb", bufs=1) as pb,
            tc.tile_pool(name="pbp", bufs=1, space="PSUM") as pbp,
        ):
            c_cur = [None, None]
            for hh in range(2):
                c_cur[hh] = pb.tile([BL, HH], F32, tag=f"c{hh}", bufs=2,
                                    name=f"c0_{hh}")
                nc.vector.memset(c_cur[hh][:], 0.0)
            # hT halves: [128, 4*BL] each, kk 0..3 in half 0, 4..7 in half 1
            hT_cur = [hT_init[:, 0:4 * BL], hT_init[:, 4 * BL:8 * BL]]

            lpend = {}

            def logits_half(p, nh):
                # logits for steps 2p, 2p+1 (M=128 tokens) from stashed
                # outsT; the two vocab halves are independent accumulation
                # groups issued on consecutive steps as PE gap filler.
                if nh == 0:
                    lt = pb.tile([128, KK_H * 128], BF16, tag="lhsT", bufs=3,
                                 name=f"lt_{p}")
                    nc.sync.dma_start(
                        lt[:].rearrange("p (k b) -> p k b", k=KK_H),
                        outsT[:, :, 128 * p:128 * (p + 1)].rearrange(
                            "k p b -> p k b"))
                    lp = pbp.tile([128, 1024], F32, tag="l_ps", bufs=1,
                                  name=f"lp_{p}")
                    lpend[p] = (lt, lp)
                lt, lp = lpend[p]
                n0, n1 = (0, 512) if nh == 0 else (512, VOCAB)
                for k in range(KK_H):
                    nc.tensor.matmul(
                        lp[:, n0:n1],
                        lhsT=lt[:, 128 * k:128 * (k + 1)],
                        rhs=linWT_sb[:, k * VOCAB + n0:k * VOCAB + n1],
                        start=(k == 0), stop=(k == KK_H - 1))
                if nh == 1:
                    del lpend[p]
                    ls = pb.tile([128, VOCAB], F32, tag="ls", bufs=2,
                                 name=f"ls_{p}")
                    nc.vector.tensor_add(ls[:], lp[:, 0:VOCAB], lb_sb[:])
                    nc.sync.dma_start(out[:, 2 * p, :], ls[0:BL])
                    nc.sync.dma_start(out[:, 2 * p + 1, :], ls[BL:128])

            def logits_pair(p):
                logits_half(p, 0)
                logits_half(p, 1)

            # stream order: g, i, f, o  (o last; g early so the c-chain
            # completes while o streams; o feeds the critical h tail).
            # colg = column base in torch gate order (for xp slicing).
            GATES = [(2048, AF.Tanh), (0, AF.Sigmoid), (1024, AF.Sigmoid),
                     (3072, AF.Sigmoid)]

            for t in range(steps):
                xp = pb.tile([BL, G4], F32, tag="xp", bufs=2)
                nc.gpsimd.indirect_dma_start(
                    out=xp[:], out_offset=None, in_=tokp,
                    in_offset=bass.IndirectOffsetOnAxis(
                        ap=caps_sb[:, t:t + 1], axis=0),
                )

                def mm_gate(psum_ap, gi, hh, k):
                    half, off = divmod(k, 4)
                    bi = (gi * 2 + hh) * KK_H + k
                    nc.tensor.matmul(
                        psum_ap,
                        lhsT=hT_cur[half][:, off * BL:(off + 1) * BL],
                        rhs=W_sb[:, bi * 512:(bi + 1) * 512],
                        start=(k == 0), stop=(k == KK_H - 1),
                    )

                # gates g,i,f: half-chunks [BL, 512]; o-gate: quarter
                # chunks [BL, 256] so its tail chain starts ~2.5us before
                # the stream ends.  One shared 1-bank psum tag for all.
                act = {}
                for gi, (colg, fn) in enumerate(GATES):
                    if gi == 3:
                        break
                    for hh in range(2):
                        gp = pbp.tile([BL, HH], F32, tag="g_ps", bufs=4,
                                      name=f"gp{gi}{hh}_{t}")
                        for k in range(KK_H):
                            mm_gate(gp[:], gi, hh, k)
                        gs = pb.tile([BL, HH], F32, tag=f"gs{gi}{hh}", bufs=1,
                                     name=f"gs{gi}{hh}_{t}")
                        nc.vector.tensor_add(
                            gs[:], gp[:],
                            xp[:, colg + HH * hh:colg + HH * (hh + 1)])
                        a = pb.tile([BL, HH], F32, tag=f"a{gi}{hh}", bufs=1,
                                    name=f"a{gi}{hh}_{t}")
                        nc.scalar.activation(a[:], gs[:], fn)
                        act[(gi, hh)] = a
                    if gi == 2:
                        # c-chain (needs g,i,f) runs while the o-gate streams
                        c_new, tc_h = [None, None], [None, None]
                        for hh in range(2):
                            t1 = pb.tile([BL, HH], F32, tag=f"t1{hh}", bufs=1,
                                         name=f"t1{hh}_{t}")
                            nc.vector.tensor_mul(
                                t1[:], act[(2, hh)][:], c_cur[hh][:])
                            t2 = pb.tile([BL, HH], F32, tag=f"t2{hh}", bufs=1,
                                         name=f"t2{hh}_{t}")
                            nc.vector.tensor_mul(
                                t2[:], act[(1, hh)][:], act[(0, hh)][:])
                            c_new[hh] = pb.tile([BL, HH], F32, tag=f"c{hh}",
                                                bufs=2, name=f"cn{hh}_{t}")
                            nc.vector.tensor_add(c_new[hh][:], t1[:], t2[:])
                            tc_h[hh] = pb.tile([BL, HH], BF16, tag=f"tc{hh}",
                                               bufs=1, name=f"tch{hh}_{t}")
                            nc.scalar.activation(
                                tc_h[hh][:], c_new[hh][:], AF.Tanh)

                # o-gate quarters + h tail: t3 = o'*tanh(c);
                # h = t3 + feature; transpose -> hT half; stash to DRAM
                QQ = HH // 2  # 256
                hT_new = [None, None]
                for hh in range(2):
                    sl = slice(HH * hh, HH * (hh + 1))
                    h = pb.tile([BL, HH], BF16, tag=f"h{hh}", bufs=2,
                                name=f"h{hh}_{t}")
                    for q in range(2):
                        qq = 2 * hh + q
                        gq = pbp.tile([BL, QQ], F32, tag="g_ps", bufs=4,
                                      name=f"gq{qq}_{t}")
                        for k in range(KK_H):
                            half, off = divmod(k, 4)
                            bi = (3 * 2 + hh) * KK_H + k
                            nc.tensor.matmul(
                                gq[:],
                                lhsT=hT_cur[half][:, off * BL:(off + 1) * BL],
                                rhs=W_sb[:, bi * 512 + QQ * q:
                                         bi * 512 + QQ * (q + 1)],
                                start=(k == 0), stop=(k == KK_H - 1),
                            )
                        go = pb.tile([BL, QQ], F32, tag=f"go{qq}", bufs=1,
                                     name=f"go{qq}_{t}")
                        nc.vector.tensor_add(
                            go[:], gq[:],
                            xp[:, 3072 + QQ * qq:3072 + QQ * (qq + 1)])
                        oa = pb.tile([BL, QQ], BF16, tag=f"oa{qq}", bufs=1,
                                     name=f"oa{qq}_{t}")
                        nc.scalar.activation(oa[:], go[:], AF.Sigmoid)
                        t3 = pb.tile([BL, QQ], BF16, tag=f"t3{qq}", bufs=1,
                                     name=f"t3{qq}_{t}")
                        nc.vector.tensor_mul(
                            t3[:], oa[:], tc_h[hh][:, QQ * q:QQ * (q + 1)])
                        nc.vector.tensor_add(
                            h[:, QQ * q:QQ * (q + 1)], t3[:],
                            feat_sb[:, HH * hh + QQ * q:
                                    HH * hh + QQ * (q + 1)])
                    hp = pbp.tile([128, 4 * BL], BF16, tag=f"h_ps{hh}", bufs=1,
                                  name=f"hp{hh}_{t}")
                    for k4 in range(4):
                        nc.tensor.matmul(
                            hp[:, k4 * BL:(k4 + 1) * BL],
                            lhsT=h[:, 128 * k4:128 * (k4 + 1)],
                            rhs=ident[:],
                            is_transpose=True,
                            start=(k4 == 0), stop=(k4 == 3),
                        )
                    hT_new[hh] = pb.tile([128, 4 * BL], BF16,
                                         tag=f"hT{hh}", bufs=2,
                                         name=f"hTn{hh}_{t}")
                    nc.vector.tensor_copy(hT_new[hh][:], hp[:])
                    nc.sync.dma_start(
                        outsT[4 * hh:4 * (hh + 1), :,
                              t * BL:(t + 1) * BL].rearrange("k p b -> p k b"),
                        hT_new[hh][:].rearrange("p (k b) -> p k b", k=4))

                hT_cur = [hT_new[0][:], hT_new[1][:]]
                c_cur = c_new

                # fill the h-tail PE gap with logits for an old step pair
                if t >= 4 and t % 2 == 0:
                    logits_pair(t // 2 - 2)

            # remaining logits pairs
            for p in range(max(0, steps // 2 - 2), (steps + 1) // 2):
                logits_pair(p)

# ---------------------------------------------------------------------------
# host glue
# ---------------------------------------------------------------------------

_CACHE = {}


def _get_program(steps=TS):
    if steps not in _CACHE:
        _CACHE[steps] = build_program(steps)
    return _CACHE[steps]


def make_in_maps(feature, captions, embed_W, W_ih, W_hh, b_ih, b_hh,
                 lin_W, lin_b):
    f32 = np.float32
    bf16 = ml_dtypes.bfloat16

    def chunkT(w, kk):
        # [R, C] -> transpose -> [kk, 128, C] -> [128, kk*C] (per-partition
        # free-dim layout: chunk-major)
        wt = np.ascontiguousarray(w.T.astype(f32))
        r = wt.reshape(kk, 128, w.shape[0])
        return np.ascontiguousarray(r.transpose(1, 0, 2).reshape(128, -1))

    embWT_p = chunkT(embed_W, KK_E)          # [128, 4*1004]
    WihT_p = chunkT(W_ih, KK_E)              # [128, 4*4096]
    blob_a = np.concatenate([embWT_p, WihT_p], axis=1).astype(bf16)

    # W_hh gate-major: block bi=(gi*2+hh)*8+k holds W_hh.T[k-chunk,
    # gate_src_col + 512*hh : +512]; stream gate order [g, i, f, o]
    wt = np.ascontiguousarray(W_hh.T.astype(f32)).reshape(KK_H, 128, G4)
    blocks = []
    for src_colg in (2048, 0, 1024, 3072):
        for hh in range(2):
            for k in range(KK_H):
                blocks.append(wt[k, :, src_colg + 512 * hh:
                                 src_colg + 512 * (hh + 1)])
    WhhT_p = np.concatenate(blocks, axis=1)       # [128, 8*4096]
    linWT_p = chunkT(lin_W, KK_H).astype(bf16)   # [128, 8*1004]

    shared = {
        "blob_a": np.ascontiguousarray(blob_a),
        "biasg": (b_ih + b_hh).astype(f32).reshape(1, G4),
        "linWT": np.ascontiguousarray(linWT_p),
        "linb": lin_b.astype(f32).reshape(1, VOCAB),
    }
    in_maps = []
    for i in range(NCORES):
        sl = slice(i * BL, (i + 1) * BL)
        fl = np.ascontiguousarray(feature[sl].astype(f32))
        featT_p = np.ascontiguousarray(
            fl.T.reshape(KK_H, 128, BL).transpose(1, 0, 2).reshape(128, -1))
        blob_b = np.concatenate([featT_p, WhhT_p], axis=1).astype(bf16)
        m = dict(shared)
        m["blob_b"] = np.ascontiguousarray(blob_b)
        m["feat"] = fl.astype(bf16)
        m["caps"] = np.ascontiguousarray(captions[sl, :TS].astype(np.int32))
        in_maps.append(m)
    return in_maps


def kernel(feature, captions, lengths=None, embed_W=None, W_ih=None,
           W_hh=None, b_ih=None, b_hh=None, lin_W=None, lin_b=None,
           trace=False):
    feature = np.asarray(feature)
    captions = np.asarray(captions)
    nc = _get_program()
    in_maps = make_in_maps(
        feature, captions, np.asarray(embed_W), np.asarray(W_ih),
        np.asarray(W_hh), np.asarray(b_ih), np.asarray(b_hh),
        np.asarray(lin_W), np.asarray(lin_b))
    res = run_bass_kernel_spmd(nc, in_maps, list(range(NCORES)), trace=trace)
    outp = np.concatenate([res.results[i]["out"] for i in range(NCORES)], axis=0)
    if trace:
        kernel.last_exec_time_ns = res.exec_time_ns
        kernel.last_results = res
    return outp



# revision 8
# speedup vs baseline: 1.5738x; 1.5738x over previous
"""Trainium2 Bass kernel for the LSTM caption decoder (nn_Decoder_62483184222858).

Math (per reference):
    emb = embed_W[captions]                      # [B, T, E]
    h0 = feature, c0 = 0
    for t in 0..T-2:
        gates = x_t @ W_ih.T + h @ W_hh.T + (b_ih+b_hh)   # [B, 4H] order i,f,g,o
        i, f, o = sigmoid(...); g = tanh(g)
        c = f*c + i*g
        h = o*tanh(c) + feature                   # emitted output AND carried state
    logits = outs @ lin_W.T + lin_b               # [B, T-1, V]

Strategy: data-parallel over 8 NeuronCores (64 batch rows each).

The recurrent matmul is computed TRANSPOSED (gatesT[4H, B] = W_hh @ h.T)
with W_hh tiles as the 128x128 stationary operand and hT chunks [128, 64]
as the moving operand: the cost-model price of a matmul is its output
free-size, so this halves the gate cost vs. streaming W_hh columns.
All elementwise state (c, h, activations) lives in chunk-major layout
[128 part, (chunk, batch)] so ops batch into full [128, 512] instructions,
and h IS the next step's moving operand (no per-step PE transposes).

Per step:
  - xp[64, 4096] gathered from the precomputed token table (phase A:
    tokp[v] = embed_W[v] @ W_ih.T + bias, bf16), then XBAR-DMA-transposed
    into chunk-major xpT [128, (gj, b)].
  - gate PSUM init via identity-matmul of xpT (start=True), then 64
    accumulating W-MMs per gate streaming hT chunks.
  - ACT sigmoid/tanh straight from PSUM; DVE c/h chain in [128, 512] ops;
    o-gate tail split in halves so h lands early.
  - logits computed per step-pair from the H4 ring (M=128), issued as PE
    gap filler at the top of each step.
"""

import sys

if "/opt/trn_rl_repo" not in sys.path:
    sys.path.insert(0, "/opt/trn_rl_repo")

import numpy as np
import ml_dtypes

import concourse.bass as bass
import concourse.mybir as mybir
import concourse.tile as tile
from concourse import bacc
from concourse.bass_utils import run_bass_kernel_spmd
from concourse.masks import make_identity

F32 = mybir.dt.float32
BF16 = mybir.dt.bfloat16
I32 = mybir.dt.int32
AF = mybir.ActivationFunctionType

EMBED, HIDDEN, VOCAB = 512, 1024, 1004
B, T = 512, 65
NCORES = 8
BL = B // NCORES          # 64 batch rows per core
TS = T - 1                # 64 time steps
G4 = 4 * HIDDEN           # 4096 gate width
KK_H = HIDDEN // 128      # 8 contraction chunks over hidden
KK_E = EMBED // 128       # 4 contraction chunks over embed
NVT = (VOCAB + 127) // 128  # 8 vocab tiles (last is 108 rows)
NGJ = G4 // 128           # 32 gate-channel tiles
HB = KK_H * BL            # 512: one h/c tile's free width (chunk-major)

# blob_a (bf16) layout: embWT | WihT  (k-chunk-major per-partition free dim)
A_EMB = 0
A_WIH = A_EMB + KK_E * VOCAB            # 4016
A_END = A_WIH + KK_E * G4               # 20400

# gate stream order (torch gate indices): g, i, f, o
GSTREAM = (2, 0, 1, 3)


def build_program(steps=TS):
    nc = bacc.Bacc("TRN2", target_bir_lowering=False, debug=False)

    blob_a = nc.dram_tensor("blob_a", [128, A_END], BF16, kind="ExternalInput")
    biasg = nc.dram_tensor("biasg", [1, G4], F32, kind="ExternalInput")
    whhT = nc.dram_tensor("whhT", [128, NGJ * KK_H * 128], BF16,
                          kind="ExternalInput")
    featT = nc.dram_tensor("featT", [128, HB], BF16, kind="ExternalInput")
    caps = nc.dram_tensor("caps", [BL, TS], I32, kind="ExternalInput")
    linWT = nc.dram_tensor("linWT", [128, KK_H * VOCAB], BF16,
                           kind="ExternalInput")
    linb = nc.dram_tensor("linb", [1, VOCAB], F32, kind="ExternalInput")
    out = nc.dram_tensor("out", [BL, TS, VOCAB], F32, kind="ExternalOutput")

    tokp = nc.dram_tensor("tokp", [VOCAB, G4], BF16, kind="Internal")

    with tile.TileContext(nc) as tc:
        _body(nc, tc, steps,
              blob_a.ap(), biasg.ap(), whhT.ap(), featT.ap(), caps.ap(),
              linWT.ap(), linb.ap(), out.ap(), tokp.ap())
    nc.compile()
    return nc


def _body(nc, tc, steps, blob_a, biasg, whhT, featT, caps, linWT, linb, out,
          tokp):
    with tc.tile_pool(name="pg", bufs=1) as pg:
        ident = pg.tile([128, 128], BF16, tag="ident")
        make_identity(nc, ident[:])

        # ================= phase A: token table =========================
        with (
            tc.tile_pool(name="pa", bufs=1) as pa,
            tc.tile_pool(name="pap", bufs=1, space="PSUM") as pap,
        ):
            ba = pa.tile([128, A_END], BF16, tag="blob_a")
            # split the load so the first k-chunks' matmuls start early
            mid = A_WIH + 2 * G4
            nc.sync.dma_start(ba[:, 0:mid], blob_a[:, 0:mid])
            nc.sync.dma_start(ba[:, mid:A_END], blob_a[:, mid:A_END])
            embWT_sb = ba[:, A_EMB:A_EMB + KK_E * VOCAB]
            WihT_sb = ba[:, A_WIH:A_WIH + KK_E * G4]
            bias_sb = pa.tile([128, G4], F32, tag="bias")
            nc.sync.dma_start(bias_sb[:], biasg.to_broadcast((128, G4)))

            # prefetch loop-phase constants (behind phase A's own loads)
            whh_sb = pg.tile([128, NGJ * KK_H * 128], BF16, tag="whh")
            for gi in GSTREAM:
                c0, c1 = gi * 8 * KK_H * 128, (gi + 1) * 8 * KK_H * 128
                nc.sync.dma_start(whh_sb[:, c0:c1], whhT[:, c0:c1])
            featT_sb = pg.tile([128, HB], BF16, tag="featT")
            nc.sync.dma_start(featT_sb[:], featT)
            caps_sb = pg.tile([BL, TS], I32, tag="caps")
            nc.sync.dma_start(caps_sb[:], caps)
            linWT_sb = pg.tile([128, KK_H * VOCAB], BF16, tag="linWT")
            nc.sync.dma_start(linWT_sb[:], linWT)
            lb_sb = pg.tile([128, VOCAB], F32, tag="lb")
            nc.sync.dma_start(lb_sb[:], linb.to_broadcast((128, VOCAB)))
            # h ring: [p, k*(4*BL) + slot*BL + b] so a step-pair's k-chunk
            # slice is contiguous (matmul lhsT needs a single free dim)
            H4 = pg.tile([128, 4 * HB], BF16, tag="H4")

            for m in range(NVT):
                mrows = min(128, VOCAB - 128 * m)
                for nb in range(G4 // 512):
                    ps = pap.tile([128, 512], F32, tag="tp_ps", bufs=8)
                    for k in range(KK_E):
                        nc.tensor.matmul(
                            ps[:mrows],
                            lhsT=embWT_sb[:, k * VOCAB + 128 * m:
                                          k * VOCAB + 128 * m + mrows],
                            rhs=WihT_sb[:, k * G4 + 512 * nb:
                                        k * G4 + 512 * (nb + 1)],
                            start=(k == 0), stop=(k == KK_E - 1),
                        )
                    sb = pa.tile([128, 512], BF16, tag="tp_sb", bufs=8)
                    nc.vector.tensor_add(
                        sb[:mrows], ps[:mrows],
                        bias_sb[:mrows, 512 * nb:512 * (nb + 1)])
                    nc.sync.dma_start(
                        tokp[128 * m:128 * m + mrows, 512 * nb:512 * (nb + 1)],
                        sb[:mrows])

        # ================= recurrence + in-loop logits ===================
        with (
            tc.tile_pool(name="pb", bufs=1) as pb,
            tc.tile_pool(name="pbp", bufs=1, space="PSUM") as pbp,
        ):
            H4v = H4[:].rearrange("p (k s b) -> p k s b", k=KK_H, s=4)

            c_cur = pb.tile([128, HB], F32, tag="c", bufs=2, name="c_init")
            nc.vector.memset(c_cur[:], 0.0)

            xp_t = {}
            xpT_t = {}

            def gather(t):
                xp_t[t] = pb.tile([BL, G4], BF16, tag="xp", bufs=3,
                                  name=f"xp_{t}")
                nc.gpsimd.indirect_dma_start(
                    out=xp_t[t][:], out_offset=None, in_=tokp,
                    in_offset=bass.IndirectOffsetOnAxis(
                        ap=caps_sb[:, t:t + 1], axis=0),
                )

            def transp(t):
                xpT_t[t] = pb.tile([128, NGJ * BL], BF16, tag="xpT", bufs=2,
                                   name=f"xpT_{t}")
                nc.sync.dma_start(
                    xpT_t[t][:].rearrange("p (j b) -> p j b", j=NGJ),
                    xp_t[t][:], transpose=True)
                del xp_t[t]

            gather(0)
            gather(1)
            transp(0)

            lp_cur = [None]

            def logits_half(p, nh):
                # logits for out steps 2p, 2p+1 from h_{2p+1}, h_{2p+2}
                # (H4 slots 2p%4, 2p%4+1 -- never wraps since 2p%4 in {0,2})
                if nh == 0:
                    lp_cur[0] = pbp.tile([128, 1024], F32, tag="lp", bufs=1,
                                         name=f"lp_{p}")
                lp = lp_cur[0]
                n0, n1 = (0, 512) if nh == 0 else (512, VOCAB)
                s0 = (2 * p) % 4
                for k in range(KK_H):
                    nc.tensor.matmul(
                        lp[:, n0:n1],
                        lhsT=H4[:, k * 4 * BL + s0 * BL:
                                k * 4 * BL + (s0 + 2) * BL],
                        rhs=linWT_sb[:, k * VOCAB + n0:k * VOCAB + n1],
                        start=(k == 0), stop=(k == KK_H - 1))

            def logits_tail(p):
                ls = pb.tile([128, VOCAB], F32, tag="ls", bufs=2,
                             name=f"ls_{p}")
                nc.vector.tensor_add(ls[:], lp_cur[0][:, 0:VOCAB], lb_sb[:])
                nc.sync.dma_start(out[:, 2 * p, :], ls[0:BL])
                nc.sync.dma_start(out[:, 2 * p + 1, :], ls[BL:128])

            for t in range(steps):
                if t + 2 < steps:
                    gather(t + 2)
                if t + 1 < steps:
                    transp(t + 1)

                if t == 0:
                    def hs(k):
                        return featT_sb[:, k * BL:(k + 1) * BL]
                else:
                    sp = (t - 1) % 4

                    def hs(k, sp=sp):
                        return H4[:, k * 4 * BL + sp * BL:
                                  k * 4 * BL + (sp + 1) * BL]

                gps = {}
                for gi in GSTREAM:
                    gps[gi] = pbp.tile([128, 512], F32, tag="gps", bufs=4,
                                       name=f"g{gi}_{t}")
                    nc.tensor.matmul(
                        gps[gi][:], lhsT=ident[:],
                        rhs=xpT_t[t][:, gi * 512:(gi + 1) * 512],
                        start=True, stop=False)
                del xpT_t[t]

                # PE gap filler: logits half-pair (independent of h_t)
                if t >= 3 and t % 2 == 1:
                    logits_half((t - 3) // 2, 0)
                elif t >= 4 and t % 2 == 0:
                    logits_half((t - 4) // 2, 1)

                # recurrent gate matmuls: W_hh tiles stationary, hT moving
                for gi in GSTREAM:
                    base = gi * 8 * KK_H * 128
                    for j in range(8):
                        oap = gps[gi][:, j * BL:(j + 1) * BL]
                        for k in range(KK_H):
                            nc.tensor.matmul(
                                oap,
                                lhsT=whh_sb[:, base + (j * KK_H + k) * 128:
                                            base + (j * KK_H + k + 1) * 128],
                                rhs=hs(k),
                                start=False, stop=(j == 7 and k == KK_H - 1),
                            )

                # activations straight from PSUM
                ag = pb.tile([128, HB], BF16, tag="ag", name=f"ag_{t}")
                nc.scalar.activation(ag[:], gps[2][:], AF.Tanh)
                ai = pb.tile([128, HB], BF16, tag="ai", name=f"ai_{t}")
                nc.scalar.activation(ai[:], gps[0][:], AF.Sigmoid)
                af = pb.tile([128, HB], BF16, tag="af", name=f"af_{t}")
                nc.scalar.activation(af[:], gps[1][:], AF.Sigmoid)

                t2 = pb.tile([128, HB], F32, tag="t2", name=f"t2_{t}")
                nc.vector.tensor_mul(t2[:], ai[:], ag[:])
                t1 = pb.tile([128, HB], F32, tag="t1", name=f"t1_{t}")
                nc.vector.tensor_mul(t1[:], af[:], c_cur[:])
                c_new = pb.tile([128, HB], F32, tag="c", bufs=2,
                                name=f"c_{t}")
                nc.vector.tensor_add(c_new[:], t1[:], t2[:])
                tcb = pb.tile([128, HB], BF16, tag="tc", name=f"tc_{t}")
                nc.scalar.activation(tcb[:], c_new[:], AF.Tanh)

                # o-gate tail in halves so the first hT chunks land early
                sw = t % 4
                for hh in range(2):
                    sl = slice(hh * 256, (hh + 1) * 256)
                    ao = pb.tile([128, 256], BF16, tag="ao", bufs=2,
                                 name=f"ao{hh}_{t}")
                    nc.scalar.activation(ao[:], gps[3][:, sl], AF.Sigmoid)
                    t3 = pb.tile([128, 256], BF16, tag="t3", bufs=2,
                                 name=f"t3{hh}_{t}")
                    nc.vector.tensor_mul(t3[:], ao[:], tcb[:, sl])
                    nc.vector.tensor_add(
                        H4v[:, hh * 4:(hh + 1) * 4, sw, :],
                        t3[:], featT_sb[:, sl])
                c_cur = c_new

                # logits epilogue late in the step (keeps DVE queue clear)
                if t >= 4 and t % 2 == 0:
                    logits_tail((t - 4) // 2)

            # trailing logits
            logits_half((steps - 4) // 2, 1)      # p=30 half B
            logits_tail((steps - 4) // 2)
            logits_half((steps - 2) // 2, 0)      # p=31
            logits_half((steps - 2) // 2, 1)
            logits_tail((steps - 2) // 2)

# ---------------------------------------------------------------------------
# host glue
# ---------------------------------------------------------------------------

_CACHE = {}


def _get_program(steps=TS):
    if steps not in _CACHE:
        _CACHE[steps] = build_program(steps)
    return _CACHE[steps]


def chunkT(w, kk):
    # [R, C] -> [128, kk*C]: [p, k*C + c] = w.T[k*128+p, c] = w[c, k*128+p]
    f32 = np.float32
    wt = np.ascontiguousarray(w.T.astype(f32))
    r = wt.reshape(kk, 128, w.shape[0])
    return np.ascontiguousarray(r.transpose(1, 0, 2).reshape(128, -1))


def make_in_maps(feature, captions, embed_W, W_ih, W_hh, b_ih, b_hh,
                 lin_W, lin_b):
    f32 = np.float32
    bf16 = ml_dtypes.bfloat16

    embWT_p = chunkT(embed_W, KK_E)          # [128, 4*1004]
    WihT_p = chunkT(W_ih, KK_E)              # [128, 4*4096]
    blob_a = np.concatenate([embWT_p, WihT_p], axis=1).astype(bf16)

    # whhT: [p, (GJ*8 + k)*128 + m] = W_hh[GJ*128 + m, k*128 + p]
    arr = W_hh.astype(f32).reshape(NGJ, 128, KK_H, 128)   # [GJ, m, k, p]
    whhT_p = np.ascontiguousarray(
        arr.transpose(3, 0, 2, 1).reshape(128, NGJ * KK_H * 128)).astype(bf16)

    linWT_p = chunkT(lin_W, KK_H).astype(bf16)   # [128, 8*1004]

    shared = {
        "blob_a": np.ascontiguousarray(blob_a),
        "biasg": (b_ih + b_hh).astype(f32).reshape(1, G4),
        "whhT": whhT_p,
        "linWT": np.ascontiguousarray(linWT_p),
        "linb": lin_b.astype(f32).reshape(1, VOCAB),
    }
    in_maps = []
    for i in range(NCORES):
        sl = slice(i * BL, (i + 1) * BL)
        fl = np.ascontiguousarray(feature[sl].astype(f32))
        featT_p = np.ascontiguousarray(
            fl.T.reshape(KK_H, 128, BL).transpose(1, 0, 2).reshape(128, HB))
        m = dict(shared)
        m["featT"] = featT_p.astype(bf16)
        m["caps"] = np.ascontiguousarray(captions[sl, :TS].astype(np.int32))
        in_maps.append(m)
    return in_maps


def kernel(feature, captions, lengths=None, embed_W=None, W_ih=None,
           W_hh=None, b_ih=None, b_hh=None, lin_W=None, lin_b=None,
           trace=False):
    feature = np.asarray(feature)
    captions = np.asarray(captions)
    nc = _get_program()
    in_maps = make_in_maps(
        feature, captions, np.asarray(embed_W), np.asarray(W_ih),
        np.asarray(W_hh), np.asarray(b_ih), np.asarray(b_hh),
        np.asarray(lin_W), np.asarray(lin_b))
    res = run_bass_kernel_spmd(nc, in_maps, list(range(NCORES)), trace=trace)
    outp = np.concatenate([res.results[i]["out"] for i in range(NCORES)], axis=0)
    if trace:
        kernel.last_exec_time_ns = res.exec_time_ns
        kernel.last_results = res
    return outp
